# revision 14
# baseline (speedup 1.0000x reference)
"""Trainium2 Bass kernel for the 3-layer GNN message-passing model.

Strategy (8 NeuronCores, SPMD):
- Nodes dst-sharded: core c owns padded rows [c*12544, (c+1)*12544); edges live
  on their destination's core.
- Aggregation algebra is folded: segment_sum distributes over the linear layers,
  so the device scatter-adds raw gathered x[src] (and edge_attr once) and applies
  one folded [97,64] matmul per layer afterwards.  LayerNorm affine params fold
  into the next layer's weights, so the device only materializes the
  normalized z.
- Scatter-add runs on the TensorEngine: each destination gets K=16 fixed
  edge slots (mean degree), so the per-tile scatter matrices are 8 constant
  block patterns; overflow edges (~10%) use small data-driven one-hot matrices
  built on the VectorEngine.
- One launch per GNN layer (3 total).  Between launches the host re-shards the
  z output into per-core gathered edge streams (pure indexing).  Pooling +
  readout MLP run on-device in launch 3 with an AllReduce for the per-graph
  sums.
"""
import sys

sys.path.insert(0, "/opt/trn_rl_repo")

import numpy as np
from ml_dtypes import bfloat16

import concourse.bacc as bacc
import concourse.mybir as mybir
import concourse.tile as tile
from concourse.bass_utils import run_bass_kernel_spmd

NC = 8
N = 100000
E = 1600000
HID = 64
ED = 16
GD = 16
G = 64
NL = 3
EPS = 1e-5

NPC = 12500
SH = 12544            # 196 * 64 = 98 * 128
NPAD = NC * SH
BLK = 64
NBLK = SH // BLK      # 196
K = 16                # main slots per dst
TPB = BLK * K // 128  # 8 main tiles per block
NTMAIN = NBLK * TPB   # 1568
MAIN_ROWS = SH * K    # 200704
SLAB = 16             # main tiles per DMA slab
OVSLAB = 8            # overflow tiles per DMA slab

F32 = mybir.dt.float32
BF16 = mybir.dt.bfloat16

TRACE = False
EXEC_NS = []          # exec_time_ns per launch when TRACE
LAST_INSTS = []       # per-launch instruction lists when TRACE


# ----------------------------------------------------------------- host prep

def _pid_of(n):
    return (n // NPC) * SH + (n % NPC)


class _St:
    pass


def _build_structure(edge_index, batch):
    src = np.asarray(edge_index[0]).astype(np.int64)
    dst = np.asarray(edge_index[1]).astype(np.int64)
    st = _St()
    spid = _pid_of(src)

    order = np.lexsort((np.arange(E), dst))
    ds = dst[order]
    cnt = np.bincount(dst, minlength=N)
    starts = np.zeros(N + 1, np.int64)
    np.cumsum(cnt, out=starts[1:])
    ranks = np.arange(E) - starts[ds]
    eids = order
    dcore = ds // NPC
    dloc = ds % NPC

    main_mask = ranks < K
    st.main_src = np.full((NC, MAIN_ROWS), -1, np.int64)
    st.main_eid = np.full((NC, MAIN_ROWS), -1, np.int64)
    slot = dloc * K + ranks
    mc, msl = dcore[main_mask], slot[main_mask]
    msrc, meid = spid[eids[main_mask]], eids[main_mask]
    for c in range(NC):
        m = mc == c
        st.main_src[c, msl[m]] = msrc[m]
        st.main_eid[c, msl[m]] = meid[m]

    ov_mask = ~main_mask
    oc = dcore[ov_mask]
    ob = dloc[ov_mask] // BLK
    o_spid = spid[eids[ov_mask]]
    o_dloc = (dloc[ov_mask] % BLK).astype(np.float32)
    o_eid = eids[ov_mask]
    # position within (core, block) group
    key = oc * NBLK + ob
    korder = np.lexsort((np.arange(len(key)), key))
    ks = key[korder]
    gcnt = np.bincount(key, minlength=NC * NBLK)
    gstart = np.zeros(NC * NBLK + 1, np.int64)
    np.cumsum(gcnt, out=gstart[1:])
    grank = np.arange(len(ks)) - gstart[ks]
    # per-block overflow tile count = max over cores
    percb = gcnt.reshape(NC, NBLK)
    st.ovt = (percb.max(axis=0) + 127) // 128
    st.NOV = max(int(st.ovt.sum()) * 128, 128)
    st.NOVT = st.NOV // 128
    block_base = np.zeros(NBLK + 1, np.int64)
    np.cumsum(st.ovt * 128, out=block_base[1:])
    st.ov_block_of_tile = np.repeat(np.arange(NBLK), st.ovt)

    st.ov_src = np.full((NC, st.NOV), -1, np.int64)
    st.ov_eid = np.full((NC, st.NOV), -1, np.int64)
    st.ov_dloc = np.zeros((NC, st.NOV), np.float32)
    pos = block_base[ks % NBLK] + grank
    core_s = ks // NBLK
    sp_s = o_spid[korder]
    dl_s = o_dloc[korder]
    ei_s = o_eid[korder]
    for c in range(NC):
        m = core_s == c
        st.ov_src[c, pos[m]] = sp_s[m]
        st.ov_eid[c, pos[m]] = ei_s[m]
        st.ov_dloc[c, pos[m]] = dl_s[m]

    batch = np.asarray(batch).astype(np.int64)
    st.batch_pad = np.full(NPAD, -1, np.int64)
    for c in range(NC):
        st.batch_pad[c * SH: c * SH + NPC] = batch[c * NPC:(c + 1) * NPC]
    st.cnt = np.bincount(batch, minlength=G).astype(np.float32)
    st.poh = np.zeros((NC, 128, SH // 128, G), np.float32)
    for c in range(NC):
        bp = st.batch_pad[c * SH:(c + 1) * SH].reshape(SH // 128, 128)
        t_idx, p_idx = np.nonzero(bp >= 0)
        st.poh[c, p_idx, t_idx, bp[t_idx, p_idx]] = 1.0
    return st


def _fold_weights(inp):
    W1, b1 = np.asarray(inp["W1"]), np.asarray(inp["b1"])
    W2, b2 = np.asarray(inp["W2"]), np.asarray(inp["b2"])
    Wu, bu = np.asarray(inp["Wu"]), np.asarray(inp["bu"])
    lnw, lnb = np.asarray(inp["lnw"]), np.asarray(inp["lnb"])
    out = []
    for i in range(NL):
        W12 = W1[i] @ W2[i][:HID]
        b12 = b1[i] @ W2[i][:HID] + b2[i]
        W2b = W2[i][HID:HID + ED]
        Wua = Wu[i][:HID]
        Wub = Wu[i][HID:HID + GD]
        lnw_p = np.ones(HID, np.float32) if i == 0 else lnw[i - 1]
        lnb_p = np.zeros(HID, np.float32) if i == 0 else lnb[i - 1]
        A = (np.diag(lnw_p) @ W12) @ Wua
        B = W2b @ Wua
        cvec = (lnb_p @ W12 + b12) @ Wua
        Wcat = np.concatenate(
            [A, B, cvec[None, :], np.zeros((15, HID), np.float32), Wub],
            0).astype(np.float32)
        out.append((np.ascontiguousarray(Wcat), bu[i].astype(np.float32)))
    return out


def _const_onehots():
    oh = np.zeros((TPB, 128, BLK), np.float32)
    for t in range(TPB):
        for p in range(128):
            oh[t, p, t * 8 + p // K] = 1.0
    return oh.transpose(1, 0, 2).reshape(128, TPB * BLK)  # [128, 8*64]


def _pad_x(x):
    xp = np.zeros((NPAD, HID), np.float32)
    for c in range(NC):
        xp[c * SH: c * SH + NPC] = x[c * NPC:(c + 1) * NPC]
    return xp


def _gather_stream(st, xfull_pad):
    main = np.zeros((NC, MAIN_ROWS, HID), bfloat16)
    ov = np.zeros((NC, st.NOV, HID), np.float32)
    for c in range(NC):
        m = st.main_src[c] >= 0
        main[c][m] = xfull_pad[st.main_src[c][m]].astype(bfloat16)
        mo = st.ov_src[c] >= 0
        ov[c][mo] = xfull_pad[st.ov_src[c][mo]]
    return main, ov


def _ea_streams(st, edge_attr):
    ea = np.asarray(edge_attr)
    main = np.zeros((NC, MAIN_ROWS, ED + 1), bfloat16)
    ov = np.zeros((NC, st.NOV, ED + 1), np.float32)
    for c in range(NC):
        m = st.main_eid[c] >= 0
        main[c][m, :ED] = ea[st.main_eid[c][m]].astype(bfloat16)
        main[c][m, ED] = 1.0
        mo = st.ov_eid[c] >= 0
        ov[c][mo, :ED] = ea[st.ov_eid[c][mo]]
        ov[c][mo, ED] = 1.0
    return main, ov


def _gpn_T(st, global_feature):
    gf = np.asarray(global_feature)
    out = np.zeros((NC, GD, SH), np.float32)
    for c in range(NC):
        bp = st.batch_pad[c * SH:(c + 1) * SH]
        m = bp >= 0
        out[c][:, m] = gf[bp[m]].T
    return out


# ------------------------------------------------------------ device program

def _build_layer_program(layer, novt, ov_block_of_tile):
    """layer: 0 (T-pass, z out, Tdeg out), 1 (Tdeg in, z out), 2 (Tdeg in,
    pooling + readout, scalar out)."""
    first = layer == 0
    last = layer == NL - 1
    EAW = ED + 1
    MW = HID + EAW if first else HID        # lhsT width main slab
    OW = HID + EAW if first else HID        # lhsT width ov slab
    PW = 81 if first else 64                # psum block width (partitions)

    nc = bacc.Bacc("TRN2", target_bir_lowering=False, debug=False)
    t_main = nc.dram_tensor("main", [MAIN_ROWS, HID], BF16, kind="ExternalInput")
    t_ov = nc.dram_tensor("ov", [max(novt * 128, 128), HID], F32, kind="ExternalInput")
    t_dstloc = nc.dram_tensor("dstloc", [128, max(novt, 1)], F32, kind="ExternalInput")
    t_ohc = nc.dram_tensor("ohc", [128, TPB * BLK], BF16, kind="ExternalInput")
    t_iota = nc.dram_tensor("iota64", [128, BLK], F32, kind="ExternalInput")
    t_gpnT = nc.dram_tensor("gpnT", [GD, SH], F32, kind="ExternalInput")
    t_wcat = nc.dram_tensor("Wcat", [112, HID], F32, kind="ExternalInput")
    t_bu = nc.dram_tensor("bu", [HID, 1], F32, kind="ExternalInput")
    if first:
        t_eamain = nc.dram_tensor("eamain", [MAIN_ROWS, EAW], BF16, kind="ExternalInput")
        t_eaov = nc.dram_tensor("eaov", [max(novt * 128, 128), EAW], F32, kind="ExternalInput")
        t_tdeg_out = nc.dram_tensor("TdegT", [17, SH], F32, kind="ExternalOutput")
    else:
        t_tdeg_in = nc.dram_tensor("TdegT_in", [17, SH], F32, kind="ExternalInput")
    t_ident = nc.dram_tensor("ident", [128, 128], F32, kind="ExternalInput")
    if not last:
        t_z = nc.dram_tensor("z", [SH, HID], F32, kind="ExternalOutput")
    else:
        t_poh = nc.dram_tensor("poh", [128, (SH // 128) * G], F32, kind="ExternalInput")
        t_scaleT = nc.dram_tensor("scaleT", [HID, G], F32, kind="ExternalInput")
        t_lnb3 = nc.dram_tensor("lnb3", [HID, 1], F32, kind="ExternalInput")
        t_gfT = nc.dram_tensor("gfT", [GD, G], F32, kind="ExternalInput")
        t_rw1 = nc.dram_tensor("rW1aug", [80, HID], F32, kind="ExternalInput")
        t_rb1t = nc.dram_tensor("rb1_t", [G, HID], F32, kind="ExternalInput")
        t_rln1w = nc.dram_tensor("rln1w_t", [G, HID], F32, kind="ExternalInput")
        t_rln1b = nc.dram_tensor("rln1b_t", [G, HID], F32, kind="ExternalInput")
        t_rw2 = nc.dram_tensor("rW2aug", [65, 32], F32, kind="ExternalInput")
        t_rln2w = nc.dram_tensor("rln2w_t", [G, 32], F32, kind="ExternalInput")
        t_rln2b = nc.dram_tensor("rln2b_t", [G, 32], F32, kind="ExternalInput")
        t_rw3 = nc.dram_tensor("rW3aug", [33, 1], F32, kind="ExternalInput")
        t_out = nc.dram_tensor("out", [G, 1], F32, kind="ExternalOutput")

    MT = nc.alloc_sbuf_tensor("MT", [112, SH], F32)

    # overflow tiles grouped per block
    ov_tiles_of_block = [[] for _ in range(NBLK)]
    for t, b in enumerate(ov_block_of_tile):
        ov_tiles_of_block[b].append(t)

    with tile.TileContext(nc) as tc:
        with (
            tc.tile_pool(name="const", bufs=1) as cp,
            tc.tile_pool(name="ms", bufs=3) as msp,
            tc.tile_pool(name="ovp", bufs=2) as ovp,
            tc.tile_pool(name="ohov", bufs=3) as ohp,
            tc.tile_pool(name="work", bufs=2) as wp,
            tc.tile_pool(name="psA", bufs=3, space="PSUM") as psA,
            tc.tile_pool(name="psB", bufs=2, space="PSUM") as psB,
            tc.tile_pool(name="psC", bufs=2, space="PSUM") as psC,
            tc.tile_pool(name="psD", bufs=1, space="PSUM") as psD,
        ):
            # ---- constants
            ohc_t = cp.tile([128, TPB * BLK], BF16, tag="ohc")
            nc.sync.dma_start(out=ohc_t[:], in_=t_ohc[:])
            iota_t = cp.tile([128, BLK], F32, tag="iota")
            nc.sync.dma_start(out=iota_t[:], in_=t_iota[:])
            dstloc_t = cp.tile([128, max(novt, 1)], F32, tag="dstloc")
            nc.sync.dma_start(out=dstloc_t[:], in_=t_dstloc[:])
            wcat_t = cp.tile([112, HID], F32, tag="wcat")
            nc.sync.dma_start(out=wcat_t[:], in_=t_wcat[:])
            bu_t = cp.tile([HID, 1], F32, tag="bu")
            nc.sync.dma_start(out=bu_t[:], in_=t_bu[:])
            ident_t = cp.tile([128, 128], F32, tag="ident")
            nc.sync.dma_start(out=ident_t[:], in_=t_ident[:])
            eps_t = cp.tile([128, 1], F32, tag="eps")
            nc.vector.memset(eps_t[:], EPS)

            nc.sync.dma_start(out=MT[96:112, :], in_=t_gpnT[:])
            if not first:
                nc.sync.dma_start(out=MT[64:81, :], in_=t_tdeg_in[:])

            # ---- scatter phase
            nslab = (NTMAIN + SLAB - 1) // SLAB
            ms_tiles = [None] * nslab
            ea_off = HID

            def load_main_slab(si):
                ntile = min(SLAB, NTMAIN - si * SLAB)
                sl = msp.tile([128, SLAB, MW], BF16, tag="ms")
                r0 = si * SLAB * 128
                nr = ntile * 128
                nc.sync.dma_start(
                    out=sl[:, :ntile, 0:HID],
                    in_=t_main[r0:r0 + nr, :].rearrange("(s p) f -> p s f", p=128),
                )
                if first:
                    nc.sync.dma_start(
                        out=sl[:, :ntile, ea_off:ea_off + EAW],
                        in_=t_eamain[r0:r0 + nr, :].rearrange("(s p) f -> p s f", p=128),
                    )
                return sl

            novslab = (novt + OVSLAB - 1) // OVSLAB if novt else 0
            ov_tiles = [None] * max(novslab, 1)

            def load_ov_slab(si):
                ntile = min(OVSLAB, novt - si * OVSLAB)
                sl = ovp.tile([128, OVSLAB, OW], F32, tag="ov")
                r0 = si * OVSLAB * 128
                nr = ntile * 128
                nc.sync.dma_start(
                    out=sl[:, :ntile, 0:HID],
                    in_=t_ov[r0:r0 + nr, :].rearrange("(s p) f -> p s f", p=128),
                )
                if first:
                    nc.sync.dma_start(
                        out=sl[:, :ntile, ea_off:ea_off + EAW],
                        in_=t_eaov[r0:r0 + nr, :].rearrange("(s p) f -> p s f", p=128),
                    )
                return sl

            for b in range(NBLK):
                ps = psA.tile([PW, BLK], F32, tag="blk")
                ovl = ov_tiles_of_block[b]
                # main MMs (bf16)
                for i in range(TPB):
                    t = b * TPB + i
                    si, sj = t // SLAB, t % SLAB
                    if ms_tiles[si] is None:
                        ms_tiles[si] = load_main_slab(si)
                    lhsT = ms_tiles[si][:, sj, 0:MW]
                    is_start = i == 0
                    is_stop = (i == TPB - 1) and not ovl
                    nc.tensor.matmul(
                        out=ps[0:PW, :],
                        lhsT=lhsT,
                        rhs=ohc_t[:, i * BLK:(i + 1) * BLK],
                        start=is_start,
                        stop=is_stop,
                    )
                # overflow MMs (f32)
                for oi, t in enumerate(ovl):
                    si, sj = t // OVSLAB, t % OVSLAB
                    if ov_tiles[si] is None:
                        ov_tiles[si] = load_ov_slab(si)
                    oh = ohp.tile([128, BLK], F32, tag="oh")
                    nc.vector.tensor_scalar(
                        out=oh[:],
                        in0=iota_t[:],
                        scalar1=dstloc_t[:, t:t + 1],
                        scalar2=None,
                        op0=mybir.AluOpType.is_equal,
                    )
                    nc.tensor.matmul(
                        out=ps[0:PW, :],
                        lhsT=ov_tiles[si][:, sj, 0:OW],
                        rhs=oh[:],
                        start=False,
                        stop=oi == len(ovl) - 1,
                    )
                nc.scalar.copy(out=MT[0:PW, b * BLK:(b + 1) * BLK], in_=ps[0:PW, :])

            if first:
                nc.sync.dma_start(out=t_tdeg_out[:], in_=MT[64:81, :])

            # ---- update + LN phase
            if last:
                poh_t = cp.tile([128, (SH // 128) * G], F32, tag="poh")
                nc.sync.dma_start(out=poh_t[:], in_=t_poh[:])
                ps_sumz = psD.tile([HID, G], F32, tag="ro")

            NJ = (SH + 511) // 512
            for j in range(NJ):
                n0 = j * 512
                nn_ = min(512, SH - n0)
                kt = nn_ // 128
                ph = psB.tile([HID, 512], F32, tag="hT")
                nc.tensor.matmul(
                    out=ph[:, :nn_], lhsT=wcat_t[:], rhs=MT[:, n0:n0 + nn_],
                    start=True, stop=True,
                )
                hts = wp.tile([HID, 512], F32, tag="hts")
                nc.vector.tensor_scalar(
                    out=hts[:, :nn_], in0=ph[:, :nn_],
                    scalar1=bu_t[:, 0:1], scalar2=None,
                    op0=mybir.AluOpType.add,
                )
                ptr = psC.tile([128, 4 * HID], F32, tag="tr")
                for kk in range(kt):
                    nc.tensor.transpose(
                        out=ptr[:, kk * HID:(kk + 1) * HID],
                        in_=hts[:, kk * 128:(kk + 1) * 128],
                        identity=ident_t[0:HID, 0:HID],
                    )
                # LN over [128, kt, 64]
                ptr3 = ptr[:, 0:kt * HID].rearrange("p (k f) -> p k f", f=HID)
                mu = wp.tile([128, 4], F32, tag="mu")
                nc.vector.tensor_reduce(
                    out=mu[:, :kt], in_=ptr3, axis=mybir.AxisListType.X,
                    op=mybir.AluOpType.add,
                )
                nc.vector.tensor_scalar(
                    out=mu[:, :kt], in0=mu[:, :kt], scalar1=1.0 / HID,
                    scalar2=None, op0=mybir.AluOpType.mult,
                )
                tb = wp.tile([128, 4, HID], F32, tag="tb")
                nc.vector.tensor_tensor(
                    out=tb[:, :kt, :], in0=ptr3,
                    in1=mu[:, :kt].to_broadcast([128, kt, HID]),
                    op=mybir.AluOpType.subtract,
                )
                sq = wp.tile([128, 4, HID], F32, tag="sq")
                nc.vector.tensor_tensor(
                    out=sq[:, :kt, :], in0=tb[:, :kt, :], in1=tb[:, :kt, :],
                    op=mybir.AluOpType.mult,
                )
                var = wp.tile([128, 4], F32, tag="var")
                nc.vector.tensor_reduce(
                    out=var[:, :kt], in_=sq[:, :kt, :], axis=mybir.AxisListType.X,
                    op=mybir.AluOpType.add,
                )
                sd = wp.tile([128, 4], F32, tag="sd")
                nc.scalar.activation(
                    out=sd[:, :kt], in_=var[:, :kt],
                    func=mybir.ActivationFunctionType.Sqrt,
                    scale=1.0 / HID, bias=eps_t[:, 0:1],
                )
                rstd = wp.tile([128, 4], F32, tag="rstd")
                nc.vector.reciprocal(out=rstd[:, :kt], in_=sd[:, :kt])
                zb = wp.tile([128, 4, HID], F32, tag="zb")
                nc.vector.tensor_tensor(
                    out=zb[:, :kt, :], in0=tb[:, :kt, :],
                    in1=rstd[:, :kt].to_broadcast([128, kt, HID]),
                    op=mybir.AluOpType.mult,
                )
                if not last:
                    nc.sync.dma_start(
                        out=t_z[n0:n0 + nn_, :].rearrange("(k p) f -> p k f", p=128),
                        in_=zb[:, :kt, :],
                    )
                else:
                    for kk in range(kt):
                        tt = j * 4 + kk
                        nc.tensor.matmul(
                            out=ps_sumz[:],
                            lhsT=zb[:, kk, :],
                            rhs=poh_t[:, tt * G:(tt + 1) * G],
                            start=tt == 0,
                            stop=tt == SH // 128 - 1,
                        )

            # ---- readout (last layer only)
            if last:
                sz = wp.tile([HID, G], F32, tag="sz")
                nc.scalar.copy(out=sz[:], in_=ps_sumz[:])
                with tc.tile_pool(name="dram", bufs=1, space="DRAM") as dp:
                    arin = dp.tile([HID, G], F32)
                    arout = dp.tile([HID, G], F32)
                    nc.gpsimd.dma_start(arin[:], sz[:])
                    nc.gpsimd.collective_compute(
                        "AllReduce",
                        mybir.AluOpType.add,
                        replica_groups=[list(range(NC))],
                        ins=[arin.opt()],
                        outs=[arout.opt()],
                    )
                    szf = wp.tile([HID, G], F32, tag="szf")
                    nc.gpsimd.dma_start(szf[:], arout[:])

                scaleT_t = cp.tile([HID, G], F32, tag="scaleT")
                nc.sync.dma_start(out=scaleT_t[:], in_=t_scaleT[:])
                lnb3_t = cp.tile([HID, 1], F32, tag="lnb3")
                nc.sync.dma_start(out=lnb3_t[:], in_=t_lnb3[:])
                gfT_t = cp.tile([GD, G], F32, tag="gfT")
                nc.sync.dma_start(out=gfT_t[:], in_=t_gfT[:])
                rw1_t = cp.tile([80, HID], F32, tag="rw1")
                nc.sync.dma_start(out=rw1_t[:], in_=t_rw1[:])
                rb1t_t = cp.tile([G, HID], F32, tag="rb1t")
                nc.sync.dma_start(out=rb1t_t[:], in_=t_rb1t[:])
                rln1w_t = cp.tile([G, HID], F32, tag="rln1w")
                nc.sync.dma_start(out=rln1w_t[:], in_=t_rln1w[:])
                rln1b_t = cp.tile([G, HID], F32, tag="rln1b")
                nc.sync.dma_start(out=rln1b_t[:], in_=t_rln1b[:])
                rw2_t = cp.tile([65, 32], F32, tag="rw2")
                nc.sync.dma_start(out=rw2_t[:], in_=t_rw2[:])
                rln2w_t = cp.tile([G, 32], F32, tag="rln2w")
                nc.sync.dma_start(out=rln2w_t[:], in_=t_rln2w[:])
                rln2b_t = cp.tile([G, 32], F32, tag="rln2b")
                nc.sync.dma_start(out=rln2b_t[:], in_=t_rln2b[:])
                rw3_t = cp.tile([33, 1], F32, tag="rw3")
                nc.sync.dma_start(out=rw3_t[:], in_=t_rw3[:])

                inT = wp.tile([80, G], F32, tag="inT")
                nc.vector.tensor_tensor(
                    out=inT[0:HID, :], in0=szf[:], in1=scaleT_t[:],
                    op=mybir.AluOpType.mult,
                )
                nc.vector.tensor_scalar(
                    out=inT[0:HID, :], in0=inT[0:HID, :],
                    scalar1=lnb3_t[:, 0:1], scalar2=None,
                    op0=mybir.AluOpType.add,
                )
                nc.vector.tensor_copy(out=inT[HID:HID + GD, :], in_=gfT_t[:])

                def ln_affine_relu(ph_in, width, w_t, b_t, out_tile):
                    """ph_in: psum [G, width] -> out_tile sbuf [G, width]"""
                    mu1 = wp.tile([G, 1], F32, tag="rmu")
                    nc.vector.tensor_reduce(
                        out=mu1[:], in_=ph_in, axis=mybir.AxisListType.X,
                        op=mybir.AluOpType.add,
                    )
                    nc.vector.tensor_scalar(
                        out=mu1[:], in0=mu1[:], scalar1=1.0 / width,
                        scalar2=None, op0=mybir.AluOpType.mult,
                    )
                    tb1 = wp.tile([G, width], F32, tag=f"rtb{width}")
                    nc.vector.tensor_scalar(
                        out=tb1[:], in0=ph_in, scalar1=mu1[:, 0:1],
                        scalar2=None, op0=mybir.AluOpType.subtract,
                    )
                    sq1 = wp.tile([G, width], F32, tag=f"rsq{width}")
                    nc.vector.tensor_tensor(
                        out=sq1[:], in0=tb1[:], in1=tb1[:],
                        op=mybir.AluOpType.mult,
                    )
                    var1 = wp.tile([G, 1], F32, tag="rvar")
                    nc.vector.tensor_reduce(
                        out=var1[:], in_=sq1[:], axis=mybir.AxisListType.X,
                        op=mybir.AluOpType.add,
                    )
                    sd1 = wp.tile([G, 1], F32, tag="rsd")
                    nc.scalar.activation(
                        out=sd1[:], in_=var1[:],
                        func=mybir.ActivationFunctionType.Sqrt,
                        scale=1.0 / width, bias=eps_t[0:G, 0:1],
                    )
                    rstd1 = wp.tile([G, 1], F32, tag="rrstd")
                    nc.vector.reciprocal(out=rstd1[:], in_=sd1[:])
                    nc.vector.tensor_scalar(
                        out=tb1[:], in0=tb1[:], scalar1=rstd1[:, 0:1],
                        scalar2=None, op0=mybir.AluOpType.mult,
                    )
                    nc.vector.tensor_tensor(
                        out=tb1[:], in0=tb1[:], in1=w_t[:],
                        op=mybir.AluOpType.mult,
                    )
                    nc.vector.tensor_tensor(
                        out=tb1[:], in0=tb1[:], in1=b_t[:],
                        op=mybir.AluOpType.add,
                    )
                    nc.vector.tensor_scalar(
                        out=out_tile[:], in0=tb1[:], scalar1=0.0,
                        scalar2=None, op0=mybir.AluOpType.max,
                    )

                ph1 = psD.tile([G, HID], F32, tag="ro")
                nc.tensor.matmul(out=ph1[:], lhsT=inT[:], rhs=rw1_t[:],
                                 start=True, stop=True)
                h1s = wp.tile([G, HID], F32, tag="h1s")
                nc.vector.tensor_tensor(out=h1s[:], in0=ph1[:], in1=rb1t_t[:],
                                        op=mybir.AluOpType.add)
                r1 = wp.tile([G, HID], F32, tag="r1")
                ln_affine_relu(h1s[:], HID, rln1w_t, rln1b_t, r1)

                ptr1 = psD.tile([G, HID], F32, tag="ro")
                nc.tensor.transpose(out=ptr1[:], in_=r1[:],
                                    identity=ident_t[0:G, 0:G])
                # r1 is [G=64, 64]; transpose -> [64, 64]
                inT2 = wp.tile([65, G], F32, tag="inT2")
                nc.scalar.copy(out=inT2[0:HID, :], in_=ptr1[:])
                nc.vector.memset(inT2[64:65, :], 1.0)

                ph2 = psD.tile([G, 32], F32, tag="ro")
                nc.tensor.matmul(out=ph2[:], lhsT=inT2[:], rhs=rw2_t[:],
                                 start=True, stop=True)
                r2 = wp.tile([G, 32], F32, tag="r2")
                ln_affine_relu(ph2[:], 32, rln2w_t, rln2b_t, r2)

                ptr2 = psD.tile([32, G], F32, tag="ro")
                nc.tensor.transpose(out=ptr2[:], in_=r2[:],
                                    identity=ident_t[0:G, 0:G])
                inT3 = wp.tile([33, G], F32, tag="inT3")
                nc.scalar.copy(out=inT3[0:32, :], in_=ptr2[:])
                nc.vector.memset(inT3[32:33, :], 1.0)

                ph3 = psD.tile([G, 1], F32, tag="ro")
                nc.tensor.matmul(out=ph3[:], lhsT=inT3[:], rhs=rw3_t[:],
                                 start=True, stop=True)
                outs = wp.tile([G, 1], F32, tag="outs")
                nc.scalar.copy(out=outs[:], in_=ph3[:])
                nc.sync.dma_start(out=t_out[:], in_=outs[:])

    nc.compile()
    return nc


# ------------------------------------------------------------------- driver

def kernel(**inputs):
    global EXEC_NS, LAST_INSTS
    EXEC_NS = []
    LAST_INSTS = []
    inputs = {k: np.asarray(v) for k, v in inputs.items()}
    st = _build_structure(inputs["edge_index"], inputs["batch"])
    folded = _fold_weights(inputs)
    ohc = _const_onehots().astype(bfloat16)
    iota64 = np.tile(np.arange(BLK, dtype=np.float32), (128, 1))
    gpnT = _gpn_T(st, inputs["global_feature"])
    xpad = _pad_x(inputs["x"].astype(np.float32))
    eam, eao = _ea_streams(st, inputs["edge_attr"])

    lnw3 = np.asarray(inputs["lnw"])[2].astype(np.float32)
    lnb3 = np.asarray(inputs["lnb"])[2].astype(np.float32)
    scaleT = np.ascontiguousarray(lnw3[:, None] / np.maximum(st.cnt, 1.0)[None, :]).astype(np.float32)
    gfT = np.ascontiguousarray(np.asarray(inputs["global_feature"]).T.astype(np.float32))
    rW1aug = np.asarray(inputs["rW1"]).astype(np.float32)
    rb1_t = np.tile(inputs["rb1"].astype(np.float32), (G, 1))
    rW2aug = np.concatenate([inputs["rW2"], inputs["rb2"][None, :]], 0).astype(np.float32)
    rW3aug = np.concatenate([inputs["rW3"], inputs["rb3"][None, :]], 0).astype(np.float32)
    rln1w_t = np.tile(inputs["rln1w"].astype(np.float32), (G, 1))
    rln1b_t = np.tile(inputs["rln1b"].astype(np.float32), (G, 1))
    rln2w_t = np.tile(inputs["rln2w"].astype(np.float32), (G, 1))
    rln2b_t = np.tile(inputs["rln2b"].astype(np.float32), (G, 1))
    ident = np.eye(128, dtype=np.float32)

    TdegT = None
    xcur = xpad
    for layer in range(NL):
        Wcat, bu = folded[layer]
        prog = _build_layer_program(layer, st.NOVT, st.ov_block_of_tile)
        main, ov = _gather_stream(st, xcur)
        in_maps = []
        for c in range(NC):
            im = {
                "main": np.ascontiguousarray(main[c]),
                "ov": np.ascontiguousarray(ov[c]),
                "dstloc": np.ascontiguousarray(
                    st.ov_dloc[c].reshape(st.NOVT, 128).T),
                "ohc": ohc,
                "iota64": iota64,
                "gpnT": np.ascontiguousarray(gpnT[c]),
                "Wcat": Wcat,
                "bu": np.ascontiguousarray(bu[:, None]),
                "ident": ident,
            }
            if layer == 0:
                im["eamain"] = np.ascontiguousarray(eam[c])
                im["eaov"] = np.ascontiguousarray(eao[c])
            else:
                im["TdegT_in"] = np.ascontiguousarray(TdegT[c])
            if layer == NL - 1:
                im["poh"] = np.ascontiguousarray(
                    st.poh[c].reshape(128, (SH // 128) * G))
                im["scaleT"] = scaleT
                im["lnb3"] = np.ascontiguousarray(lnb3[:, None])
                im["gfT"] = gfT
                im["rW1aug"] = rW1aug
                im["rb1_t"] = rb1_t
                im["rln1w_t"] = rln1w_t
                im["rln1b_t"] = rln1b_t
                im["rW2aug"] = rW2aug
                im["rln2w_t"] = rln2w_t
                im["rln2b_t"] = rln2b_t
                im["rW3aug"] = rW3aug
            in_maps.append(im)
        res = run_bass_kernel_spmd(
            prog, in_maps, core_ids=list(range(NC)), trace=TRACE)
        if TRACE:
            EXEC_NS.append(res.exec_time_ns)
            LAST_INSTS.append(res.instructions_and_trace[0] if res.instructions_and_trace else None)
        if layer == 0:
            TdegT = [res.results[c]["TdegT"] for c in range(NC)]
        if layer < NL - 1:
            xnext = np.zeros((NPAD, HID), np.float32)
            for c in range(NC):
                xnext[c * SH:(c + 1) * SH] = res.results[c]["z"]
            xcur = xnext
        else:
            out = res.results[0]["out"][:, 0].astype(np.float32)
    return out
